# revision 12
# baseline (speedup 1.0000x reference)
"""Trainium2 Bass kernel: per-pixel channel shuffle + 3x3 conv (stride 1, pad 1).

Problem: x [32,256,56,56] f32, w [256,256,3,3] f32 (OIHW), perm [3136,256] i32;
out[b,:,h,w] = conv3x3(xs)[b,:,h,w] where xs[b,:,l] = x[b, perm[l,:], l].

Strategy (8 NeuronCores, data-parallel over batch, 4 batches/core):
  host: pre-transpose x to [B, HW, C] bf16; build inverse-perm int16 tables
        batched 4 pixel-tiles per GPSIMD local_scatter call; pre-transform w
        into 48 F(2,3)-Winograd lhsT tiles U[i,kh] (width-only Winograd:
        2 output cols from 4 input cols, 4 muls instead of 6).
  device, per batch (pipelined across engines):
    step s (7 per batch): GPSIMD local_scatter shuffles 448 pixels x 256 ch;
    PE transposes back to [c, l] (8 x [112,128] identity matmuls -> PSUM);
    Scalar copies the 8 rows into a zero-padded 58x58 flat image xs; DVE
    computes the width-Winograd input transform V[i][R, tc] (3 ops/ct, the
    i=0/i=3 planes paired in one strided op). Conv for row-group g=s-2:
    24 matmuls (4 wino-i x 3 kh x 2 ic-tiles) of N=224 accumulate into 4
    PSUM slices M0..M3; Scalar stages M1,M2 to SBUF; DVE computes
    Y_even = M0+M1+M2, Y_odd = M1-M2-M3 and writes the 448-col output
    staging tile, DMA'd out contiguously.
"""

import os
import sys
import types
import numpy as np

_STATE = {}
LAST_RESULT = None

B, C, H, W = 32, 256, 56, 56
HW = H * W
PADW = 58
XS_LEN = 3376
TL = 112
NT = 28
NSC = 7          # scatter steps per batch (4 pixel-tiles each)
K4 = 4           # pixel-tiles per local_scatter call
VPL = 58 * 28    # one V plane: 58 padded rows x 28 tile-cols
N_GROUPS = 7
N_CORES = 8
B_LOC = B // N_CORES
# V plane order in memory: [i0, i3, i1, i2] so the two "subtract" planes
# (i0 = d0-d2, i3 = d1-d3) sit adjacent and are computed by one paired op.
PLANE = {0: 0, 3: 1, 1: 2, 2: 3}


def _install_ntff_shim():
    # antenv.axon_hooks is absent in some images; provide it so trace=True
    # (BASS_TRACE=1) can capture NTFF profiles instead of crashing.
    name = "antenv.axon_hooks"
    if name in sys.modules:
        return
    try:
        import antenv  # noqa: F401

        m = types.ModuleType(name)
        m._hook = None
        m.set_axon_ntff_profile_hook = lambda h: setattr(m, "_hook", h)
        m.get_axon_ntff_profile_hook = lambda: m._hook
        sys.modules[name] = m
        setattr(sys.modules["antenv"], "axon_hooks", m)
        from trn_agent_boot.trn_boot import _ntff_profile_via_ctypes

        hook = _ntff_profile_via_ctypes("/opt/axon/libaxon_pjrt.so")
        if hook is not None:
            m.set_axon_ntff_profile_hook(hook)
    except Exception:
        pass


def _build_kernel():
    import concourse.bass as bass
    import concourse.mybir as mybir
    from concourse import bacc, tile
    from concourse.masks import make_identity
    from contextlib import ExitStack

    F32 = mybir.dt.float32
    BF16 = mybir.dt.bfloat16
    I16 = mybir.dt.int16

    nc = bacc.Bacc("TRN2", target_bir_lowering=False, debug=False, num_devices=N_CORES)

    xb = nc.dram_tensor("xb", [B_LOC, HW, C], BF16, kind="ExternalInput")
    wt = nc.dram_tensor("wt", [48, 128, 128], BF16, kind="ExternalInput")
    idxt = nc.dram_tensor("idxt", [128, NSC * K4 * 256], I16, kind="ExternalInput")
    out = nc.dram_tensor("out", [B_LOC, C, HW], F32, kind="ExternalOutput")

    def sub_ap(ap, off, dims):
        # hand-built sub-AP of an SBUF tile slice: keep the partition dim,
        # replace the free dims.
        return bass.AP(ap.tensor, ap.offset + off, [ap.ap[0]] + dims)

    with tile.TileContext(nc) as tc, ExitStack() as ctx:
        const = ctx.enter_context(tc.tile_pool(name="const", bufs=1))
        wsb = const.tile([128, 48 * 128], BF16)
        idxsb = const.tile([128, NSC * 1024], I16)
        ident = const.tile([128, 128], BF16)
        make_identity(nc, ident[:, :])
        nc.sync.dma_start(out=idxsb[:, 0:1024], in_=idxt[:, 0:1024])
        nc.scalar.dma_start(
            out=wsb[:, :],
            in_=bass.AP(wt, 0, [[128, 128], [128 * 128, 48], [1, 128]]),
        )

        xin_pool = ctx.enter_context(tc.tile_pool(name="xin", bufs=2))
        xs_pool = ctx.enter_context(tc.tile_pool(name="xs", bufs=2))
        v_pool = ctx.enter_context(tc.tile_pool(name="vv", bufs=2))
        sout_pool = ctx.enter_context(tc.tile_pool(name="sout", bufs=4))
        outst_pool = ctx.enter_context(tc.tile_pool(name="outst", bufs=4))
        mst_pool = ctx.enter_context(tc.tile_pool(name="mst", bufs=4))
        tsc_pool = ctx.enter_context(tc.tile_pool(name="tsc", bufs=4))
        tps_pool = ctx.enter_context(tc.tile_pool(name="tps", bufs=2, space="PSUM"))
        mpsum_pool = ctx.enter_context(tc.tile_pool(name="mpsum", bufs=3, space="PSUM"))

        xin_t = [None] * B_LOC

        def issue_xin_dma(b, chunks=1):
            xin_t[b] = xin_pool.tile([128, NT * 256], BF16, name="xin", tag="xin")
            step = NSC // chunks
            for s in range(0, NSC, step):
                nc.sync.dma_start(
                    out=xin_t[b][0:TL, s * 1024 : (s + step) * 1024],
                    in_=bass.AP(
                        xb,
                        b * HW * C + s * K4 * TL * 256,
                        [[256, TL], [TL * 256, step * K4], [1, 256]],
                    ),
                )

        def wconv_group(b, vt, oct, g):
            mp = mpsum_pool.tile([128, 1024], F32)
            for i in range(4):
                for kh in range(3):
                    for ct in range(2):
                        widx = ((i * 3 + kh) * 2 + ct) * 2 + oct
                        voff = (ct * 4 + PLANE[i]) * VPL + (8 * g + kh) * 28
                        nc.tensor.matmul(
                            mp[:, i * 256 : i * 256 + 224],
                            lhsT=wsb[:, widx * 128 : (widx + 1) * 128],
                            rhs=vt[:, voff : voff + 224],
                            start=(kh == 0 and ct == 0),
                            stop=(kh == 2 and ct == 1),
                        )
            # stage M1, M2 to SBUF (bf16) so the DVE inverse-transform ops
            # each touch at most one PSUM operand (ISA limit).
            ms = mst_pool.tile([128, 448], BF16)
            nc.scalar.copy(ms[:, 0:224], mp[:, 256:480])
            nc.scalar.copy(ms[:, 224:448], mp[:, 512:736])
            t01 = tsc_pool.tile([128, 448], BF16)
            nc.vector.tensor_add(t01[:, 0:224], mp[:, 0:224], ms[:, 0:224])
            nc.vector.tensor_add(t01[:, 224:448], mp[:, 768:992], ms[:, 224:448])
            ost = outst_pool.tile([128, 448], F32)
            oap = ost[:, :]
            even = sub_ap(oap, 0, [[56, 8], [2, 28]])
            odd = sub_ap(oap, 1, [[56, 8], [2, 28]])
            r28 = lambda ap: ap.rearrange("p (r c) -> p r c", r=8)
            nc.vector.tensor_add(even, r28(t01[:, 0:224]), r28(mp[:, 512:736]))
            nc.vector.tensor_sub(odd, r28(ms[:, 0:224]), r28(t01[:, 224:448]))
            nc.sync.dma_start(
                out=out[b, oct * 128 : (oct + 1) * 128, g * 448 : (g + 1) * 448],
                in_=ost[:, :],
            )

        issue_xin_dma(0, chunks=7)
        for s in range(1, NSC):
            nc.sync.dma_start(
                out=idxsb[:, s * 1024 : (s + 1) * 1024],
                in_=idxt[:, s * 1024 : (s + 1) * 1024],
            )
        for b in range(B_LOC):
            if b + 1 < B_LOC:
                issue_xin_dma(b + 1)
            xin = xin_t[b]

            xs = xs_pool.tile([128, 2 * XS_LEN], BF16, name="xs", tag="xs")
            vt = v_pool.tile([128, 8 * VPL], BF16, name="vt", tag="vt")
            for ct in range(2):
                base = ct * XS_LEN
                nc.vector.memset(xs[:, base : base + PADW], 0.0)
                nc.vector.memset(xs[:, base + 57 * PADW : base + XS_LEN], 0.0)
                nc.vector.memset(
                    xs[:, base + PADW : base + PADW + 56 * PADW].rearrange(
                        "p (r x) -> p r x", r=56
                    )[:, :, 0:1],
                    0.0,
                )
                nc.vector.memset(
                    xs[:, base + PADW + 57 : base + PADW + 57 + 56 * PADW].rearrange(
                        "p (r x) -> p r x", r=56
                    )[:, :, 0:1],
                    0.0,
                )

            for s in range(NSC):
                if s >= 2:
                    wconv_group(b, vt, 0, s - 2)
                    wconv_group(b, vt, 1, s - 2)
                sout = sout_pool.tile([128, K4 * 256], BF16, name="sout", tag="sout")
                nc.gpsimd.local_scatter(
                    out_ap=sout[0:TL, :],
                    data_ap=xin[0:TL, s * 1024 : (s + 1) * 1024],
                    idxs_ap=idxsb[0:TL, s * 1024 : (s + 1) * 1024],
                    channels=TL,
                    num_elems=K4 * 256,
                    num_idxs=K4 * 256,
                )
                ps2 = tps_pool.tile([128, 2 * 448], BF16, name="ps2", tag="ps2")
                for ct in range(2):
                    for k in range(K4):
                        nc.tensor.transpose(
                            ps2[:, ct * 448 + k * TL : ct * 448 + (k + 1) * TL],
                            sout[0:TL, k * 256 + ct * 128 : k * 256 + ct * 128 + 128],
                            ident[0:TL, 0:TL],
                        )
                q = 59 + 8 * s * PADW
                for ct in range(2):
                    nc.scalar.copy(
                        xs[:, ct * XS_LEN + q : ct * XS_LEN + q + 8 * PADW].rearrange(
                            "p (r x) -> p r x", r=8
                        )[:, :, 0:56],
                        ps2[:, ct * 448 : (ct + 1) * 448].rearrange(
                            "p (r x) -> p r x", r=8
                        ),
                    )
                # width-Winograd input transform for the rows just written
                # (plus the top/bottom zero pad rows at s=0 / s=6):
                # V[i0] = d0-d2, V[i3] = d1-d3 (paired), V[i1] = d1+d2,
                # V[i2] = d2-d1.
                if s == 0:
                    r0, nr = 0, 9
                elif s == 6:
                    r0, nr = 49, 9
                else:
                    r0, nr = 8 * s + 1, 8
                xa = xs[:, :]
                va = vt[:, :]
                for ct in range(2):
                    xoff = ct * XS_LEN + r0 * PADW
                    vof = ct * 4 * VPL + r0 * 28
                    nc.vector.tensor_sub(
                        sub_ap(va, vof, [[VPL, 2], [28, nr], [1, 28]]),
                        sub_ap(xa, xoff + 0, [[1, 2], [PADW, nr], [2, 28]]),
                        sub_ap(xa, xoff + 2, [[1, 2], [PADW, nr], [2, 28]]),
                    )
                    nc.vector.tensor_add(
                        sub_ap(va, vof + 2 * VPL, [[28, nr], [1, 28]]),
                        sub_ap(xa, xoff + 1, [[PADW, nr], [2, 28]]),
                        sub_ap(xa, xoff + 2, [[PADW, nr], [2, 28]]),
                    )
                    nc.vector.tensor_sub(
                        sub_ap(va, vof + 3 * VPL, [[28, nr], [1, 28]]),
                        sub_ap(xa, xoff + 2, [[PADW, nr], [2, 28]]),
                        sub_ap(xa, xoff + 1, [[PADW, nr], [2, 28]]),
                    )

            for g in (5, 6):
                wconv_group(b, vt, 0, g)
                wconv_group(b, vt, 1, g)

    nc.compile()
    return nc


def _host_prep(x, w, perm):
    import ml_dtypes

    # [B, C, H, W] -> [B, HW, C] bf16 (pixel-major so scatter tiles DMA
    # straight into [pixel, channel] layout)
    xf = np.ascontiguousarray(
        x.reshape(B, C, HW).transpose(0, 2, 1)
    ).astype(ml_dtypes.bfloat16)

    # F(2,3) width-Winograd weight transform: U0 = g0, U1 = (g0+g1+g2)/2,
    # U2 = (g0-g1+g2)/2, U3 = g2 per (kh, ic-tile, oc-tile), stored as lhsT.
    wf = np.asarray(w, dtype=np.float32)
    wtl = np.empty((48, 128, 128), dtype=ml_dtypes.bfloat16)
    for ct in range(2):
        for kh in range(3):
            for oct in range(2):
                blk = wf[
                    oct * 128 : (oct + 1) * 128, ct * 128 : (ct + 1) * 128, kh, :
                ]  # [oc, ic, 3]
                g0, g1, g2 = blk[:, :, 0], blk[:, :, 1], blk[:, :, 2]
                U = [g0, 0.5 * (g0 + g1 + g2), 0.5 * (g0 - g1 + g2), g2]
                for i in range(4):
                    widx = ((i * 3 + kh) * 2 + ct) * 2 + oct
                    wtl[widx] = U[i].T.astype(ml_dtypes.bfloat16)

    iperm = np.empty((HW, C), dtype=np.int16)
    np.put_along_axis(
        iperm, perm.astype(np.int64), np.arange(C, dtype=np.int16)[None, :], axis=1
    )
    idxt = np.zeros((128, NSC * 1024), dtype=np.int16)
    for s in range(NSC):
        for k in range(K4):
            t = K4 * s + k
            idxt[0:TL, s * 1024 + k * 256 : s * 1024 + (k + 1) * 256] = (
                iperm[t * TL : (t + 1) * TL, :] + k * 256
            )

    in_maps = []
    for cidx in range(N_CORES):
        in_maps.append(
            {
                "xb": np.ascontiguousarray(xf[cidx * B_LOC : (cidx + 1) * B_LOC]),
                "wt": wtl,
                "idxt": idxt,
            }
        )
    return in_maps


def kernel(x, w, perm):
    global LAST_RESULT
    _install_ntff_shim()
    from concourse.bass_utils import run_bass_kernel_spmd

    x = np.asarray(x, dtype=np.float32)
    w = np.asarray(w, dtype=np.float32)
    perm = np.asarray(perm)

    if "nc" not in _STATE:
        _STATE["nc"] = _build_kernel()
    nc = _STATE["nc"]

    in_maps = _host_prep(x, w, perm)
    res = run_bass_kernel_spmd(nc, in_maps, core_ids=list(range(N_CORES)))
    LAST_RESULT = res
    out = np.concatenate(
        [r["out"].reshape(B_LOC, C, H, W) for r in res.results], axis=0
    )
    return out.astype(np.float32)


# revision 15
# speedup vs baseline: 1.1675x; 1.1675x over previous
"""Trainium2 Bass kernel: per-pixel channel shuffle + 3x3 conv (stride 1, pad 1).

Problem: x [32,256,56,56] f32, w [256,256,3,3] f32 (OIHW), perm [3136,256] i32;
out[b,:,h,w] = conv3x3(xs)[b,:,h,w] where xs[b,:,l] = x[b, perm[l,:], l].

Strategy (8 NeuronCores, data-parallel over batch, 4 batches/core), using
width-only F(2,3) Winograd (2 output cols from 4 input cols, 4 muls vs 6):
  host: pre-transpose x to [B, HW, C] bf16; build inverse-perm int16 tables
        batched 4 pixel-tiles per GPSIMD local_scatter call; transform w into
        48 Winograd lhsT tiles U[i,kh]; build the 112x224 B-transform matrix
        T that maps a 112-pixel tile (2 image rows) to its 4 Winograd input
        planes x 2 rows x 28 tile-cols (boundary columns dropped = padding).
  device, per batch (pipelined across engines):
    step s (7 per batch): GPSIMD local_scatter shuffles 448 pixels x 256 ch;
    PE multiplies each [112,128] block by T — transposing back to channel-
    major AND computing the Winograd input transform in one matmul -> PSUM;
    Scalar copies the V planes to SBUF. Conv for row-group g=s-2: 24 matmuls
    (4 wino-i x 3 kh x 2 ic-tiles) of N=224 accumulate into 4 PSUM slices
    M0..M3; Scalar stages M1; DVE computes Y_even = (M0+M1)+M2 and
    Y_odd = (M1-M2)-M3 into the w-interleaved staging tile, DMA'd out.
"""

import os
import sys
import types
import numpy as np

_STATE = {}
LAST_RESULT = None

B, C, H, W = 32, 256, 56, 56
HW = H * W
TL = 112
NT = 28
NSC = 7          # scatter steps per batch (4 pixel-tiles each)
K4 = 4           # pixel-tiles per local_scatter call
VPL = 58 * 28    # one V plane: 58 padded rows x 28 tile-cols
N_CORES = 8
B_LOC = B // N_CORES
# Winograd i -> V plane slot (memory order i0,i3,i1,i2 — legacy of the DVE
# variant; any consistent order works since T and the conv agree on it).
PLANE = {0: 0, 3: 1, 1: 2, 2: 3}


def _install_ntff_shim():
    # antenv.axon_hooks is absent in some images; provide it so trace=True
    # (BASS_TRACE=1) can capture NTFF profiles instead of crashing.
    name = "antenv.axon_hooks"
    if name in sys.modules:
        return
    try:
        import antenv  # noqa: F401

        m = types.ModuleType(name)
        m._hook = None
        m.set_axon_ntff_profile_hook = lambda h: setattr(m, "_hook", h)
        m.get_axon_ntff_profile_hook = lambda: m._hook
        sys.modules[name] = m
        setattr(sys.modules["antenv"], "axon_hooks", m)
        from trn_agent_boot.trn_boot import _ntff_profile_via_ctypes

        hook = _ntff_profile_via_ctypes("/opt/axon/libaxon_pjrt.so")
        if hook is not None:
            m.set_axon_ntff_profile_hook(hook)
    except Exception:
        pass


def _build_kernel():
    import concourse.bass as bass
    import concourse.mybir as mybir
    from concourse import bacc, tile
    from contextlib import ExitStack

    F32 = mybir.dt.float32
    BF16 = mybir.dt.bfloat16
    I16 = mybir.dt.int16

    nc = bacc.Bacc("TRN2", target_bir_lowering=False, debug=False, num_devices=N_CORES)

    xb = nc.dram_tensor("xb", [B_LOC, HW, C], BF16, kind="ExternalInput")
    wt = nc.dram_tensor("wt", [48, 128, 128], BF16, kind="ExternalInput")
    tmat = nc.dram_tensor("tmat", [TL, 224], BF16, kind="ExternalInput")
    idxt = nc.dram_tensor("idxt", [128, NSC * K4 * 256], I16, kind="ExternalInput")
    out = nc.dram_tensor("out", [B_LOC, C, HW], F32, kind="ExternalOutput")

    def sub_ap(ap, off, dims):
        # hand-built sub-AP of an SBUF/PSUM tile slice: keep the partition
        # dim, replace the free dims.
        return bass.AP(ap.tensor, ap.offset + off, [ap.ap[0]] + dims)

    with tile.TileContext(nc) as tc, ExitStack() as ctx:
        const = ctx.enter_context(tc.tile_pool(name="const", bufs=1))
        wsb = const.tile([128, 48 * 128], BF16)
        idxsb = const.tile([128, NSC * 1024], I16)
        tsb = const.tile([128, 224], BF16)
        nc.sync.dma_start(out=idxsb[:, 0:1024], in_=idxt[:, 0:1024])
        nc.sync.dma_start(out=tsb[0:TL, :], in_=tmat[:, :])
        nc.scalar.dma_start(
            out=wsb[:, :],
            in_=bass.AP(wt, 0, [[128, 128], [128 * 128, 48], [1, 128]]),
        )

        xin_pool = ctx.enter_context(tc.tile_pool(name="xin", bufs=2))
        v_pool = ctx.enter_context(tc.tile_pool(name="vv", bufs=2))
        sout_pool = ctx.enter_context(tc.tile_pool(name="sout", bufs=4))
        outst_pool = ctx.enter_context(tc.tile_pool(name="outst", bufs=4))
        mst_pool = ctx.enter_context(tc.tile_pool(name="mst", bufs=4))
        tsc_pool = ctx.enter_context(tc.tile_pool(name="tsc", bufs=4))
        tps_pool = ctx.enter_context(tc.tile_pool(name="tps", bufs=1, space="PSUM"))
        mpsum_pool = ctx.enter_context(tc.tile_pool(name="mpsum", bufs=2, space="PSUM"))

        xin_t = [None] * B_LOC

        def issue_xin_dma(b, chunks=1):
            xin_t[b] = xin_pool.tile([128, NT * 256], BF16, name="xin", tag="xin")
            step = NSC // chunks
            for s in range(0, NSC, step):
                nc.sync.dma_start(
                    out=xin_t[b][0:TL, s * 1024 : (s + step) * 1024],
                    in_=bass.AP(
                        xb,
                        b * HW * C + s * K4 * TL * 256,
                        [[256, TL], [TL * 256, step * K4], [1, 256]],
                    ),
                )

        def wconv_group(b, vt, oct, g):
            mp = mpsum_pool.tile([128, 1024], F32)
            for i in range(4):
                for kh in range(3):
                    for ct in range(2):
                        widx = ((i * 3 + kh) * 2 + ct) * 2 + oct
                        voff = (ct * 4 + PLANE[i]) * VPL + (8 * g + kh) * 28
                        nc.tensor.matmul(
                            mp[:, i * 256 : i * 256 + 224],
                            lhsT=wsb[:, widx * 128 : (widx + 1) * 128],
                            rhs=vt[:, voff : voff + 224],
                            start=(kh == 0 and ct == 0),
                            stop=(kh == 2 and ct == 1),
                        )
            # stage M1 to SBUF (bf16) so every DVE inverse op touches at
            # most one PSUM operand (ISA limit).
            ms = mst_pool.tile([128, 224], BF16)
            nc.scalar.copy(ms[:, :], mp[:, 256:480])
            t01 = tsc_pool.tile([128, 448], BF16)
            nc.vector.tensor_add(t01[:, 0:224], mp[:, 0:224], ms[:, :])
            nc.vector.tensor_sub(t01[:, 224:448], ms[:, :], mp[:, 512:736])
            ost = outst_pool.tile([128, 448], F32)
            oap = ost[:, :]
            even = sub_ap(oap, 0, [[56, 8], [2, 28]])
            odd = sub_ap(oap, 1, [[56, 8], [2, 28]])
            r28 = lambda ap: ap.rearrange("p (r c) -> p r c", r=8)
            nc.vector.tensor_add(even, r28(t01[:, 0:224]), r28(mp[:, 512:736]))
            nc.vector.tensor_sub(odd, r28(t01[:, 224:448]), r28(mp[:, 768:992]))
            nc.sync.dma_start(
                out=out[b, oct * 128 : (oct + 1) * 128, g * 448 : (g + 1) * 448],
                in_=ost[:, :],
            )

        issue_xin_dma(0, chunks=7)
        for s in range(1, NSC):
            nc.sync.dma_start(
                out=idxsb[:, s * 1024 : (s + 1) * 1024],
                in_=idxt[:, s * 1024 : (s + 1) * 1024],
            )
        for b in range(B_LOC):
            if b + 1 < B_LOC:
                issue_xin_dma(b + 1)
            xin = xin_t[b]

            vt = v_pool.tile([128, 8 * VPL], BF16, name="vt", tag="vt")
            va = vt[:, :]
            # top/bottom padding rows of every V plane are zero
            for ct in range(2):
                for row in (0, 57):
                    nc.vector.memset(
                        sub_ap(va, ct * 4 * VPL + row * 28, [[VPL, 4], [1, 28]]),
                        0.0,
                    )

            for s in range(NSC):
                if s >= 2:
                    wconv_group(b, vt, 0, s - 2)
                    wconv_group(b, vt, 1, s - 2)
                sout = sout_pool.tile([128, K4 * 256], BF16, name="sout", tag="sout")
                nc.gpsimd.local_scatter(
                    out_ap=sout[0:TL, :],
                    data_ap=xin[0:TL, s * 1024 : (s + 1) * 1024],
                    idxs_ap=idxsb[0:TL, s * 1024 : (s + 1) * 1024],
                    channels=TL,
                    num_elems=K4 * 256,
                    num_idxs=K4 * 256,
                )
                # per [112,128] block: transpose-back x Winograd-B in one
                # matmul into f32 PSUM; block j = ct*4+k at a bank-safe
                # offset (two 224-col blocks per 512-f32 bank)
                ps2 = tps_pool.tile([128, 2048], F32, name="ps2", tag="ps2")
                for ct in range(2):
                    for k in range(K4):
                        j = ct * K4 + k
                        off = (j // 2) * 512 + (j % 2) * 224
                        nc.tensor.matmul(
                            ps2[:, off : off + 224],
                            lhsT=sout[0:TL, k * 256 + ct * 128 : k * 256 + ct * 128 + 128],
                            rhs=tsb[0:TL, :],
                            start=True,
                            stop=True,
                        )
                # copy the step's V rows (8s+1 .. 8s+8) into the V planes,
                # one copy per (ct, pair-of-tiles)
                pa = ps2[:, :]
                for ct in range(2):
                    for k2 in range(2):
                        nc.scalar.copy(
                            sub_ap(
                                va,
                                ct * 4 * VPL + (8 * s + 1 + 4 * k2) * 28,
                                [[VPL, 4], [56, 2], [28, 2], [1, 28]],
                            ),
                            sub_ap(
                                pa,
                                ct * 1024 + k2 * 512,
                                [[56, 4], [224, 2], [28, 2], [1, 28]],
                            ),
                        )

            for g in (5, 6):
                wconv_group(b, vt, 0, g)
                wconv_group(b, vt, 1, g)

    nc.compile()
    return nc


def _host_prep(x, w, perm):
    import ml_dtypes

    # [B, C, H, W] -> [B, HW, C] bf16 (pixel-major so scatter tiles DMA
    # straight into [pixel, channel] layout)
    xf = np.ascontiguousarray(
        x.reshape(B, C, HW).transpose(0, 2, 1)
    ).astype(ml_dtypes.bfloat16)

    # F(2,3) width-Winograd weight transform: U0 = g0, U1 = (g0+g1+g2)/2,
    # U2 = (g0-g1+g2)/2, U3 = g2 per (kh, ic-tile, oc-tile), stored as lhsT.
    wf = np.asarray(w, dtype=np.float32)
    wtl = np.empty((48, 128, 128), dtype=ml_dtypes.bfloat16)
    for ct in range(2):
        for kh in range(3):
            for oct in range(2):
                blk = wf[
                    oct * 128 : (oct + 1) * 128, ct * 128 : (ct + 1) * 128, kh, :
                ]  # [oc, ic, 3]
                g0, g1, g2 = blk[:, :, 0], blk[:, :, 1], blk[:, :, 2]
                U = [g0, 0.5 * (g0 + g1 + g2), 0.5 * (g0 - g1 + g2), g2]
                for i in range(4):
                    widx = ((i * 3 + kh) * 2 + ct) * 2 + oct
                    wtl[widx] = U[i].T.astype(ml_dtypes.bfloat16)

    # B-transform matrix: block pixel p = (r_loc, w); output col =
    # plane*56 + r_loc*28 + tc. V[i][tc] = sum_a Bcoef[i][a] * d(2tc+a-1)
    # where d is the image column (pad columns outside [0,56) drop out).
    BCOEF = {0: {0: 1.0, 2: -1.0}, 1: {1: 1.0, 2: 1.0},
             2: {2: 1.0, 1: -1.0}, 3: {1: 1.0, 3: -1.0}}
    tmat = np.zeros((TL, 224), dtype=np.float32)
    for i in range(4):
        pl = PLANE[i]
        for tc in range(28):
            for a, cf in BCOEF[i].items():
                wcol = 2 * tc + a - 1
                if 0 <= wcol < 56:
                    for r_loc in range(2):
                        tmat[r_loc * 56 + wcol, pl * 56 + r_loc * 28 + tc] = cf
    tmat = tmat.astype(ml_dtypes.bfloat16)

    iperm = np.empty((HW, C), dtype=np.int16)
    np.put_along_axis(
        iperm, perm.astype(np.int64), np.arange(C, dtype=np.int16)[None, :], axis=1
    )
    idxt = np.zeros((128, NSC * 1024), dtype=np.int16)
    for s in range(NSC):
        for k in range(K4):
            t = K4 * s + k
            idxt[0:TL, s * 1024 + k * 256 : s * 1024 + (k + 1) * 256] = (
                iperm[t * TL : (t + 1) * TL, :] + k * 256
            )

    in_maps = []
    for cidx in range(N_CORES):
        in_maps.append(
            {
                "xb": np.ascontiguousarray(xf[cidx * B_LOC : (cidx + 1) * B_LOC]),
                "wt": wtl,
                "tmat": tmat,
                "idxt": idxt,
            }
        )
    return in_maps


def kernel(x, w, perm):
    global LAST_RESULT
    _install_ntff_shim()
    from concourse.bass_utils import run_bass_kernel_spmd

    x = np.asarray(x, dtype=np.float32)
    w = np.asarray(w, dtype=np.float32)
    perm = np.asarray(perm)

    if "nc" not in _STATE:
        _STATE["nc"] = _build_kernel()
    nc = _STATE["nc"]

    in_maps = _host_prep(x, w, perm)
    res = run_bass_kernel_spmd(nc, in_maps, core_ids=list(range(N_CORES)))
    LAST_RESULT = res
    out = np.concatenate(
        [r["out"].reshape(B_LOC, C, H, W) for r in res.results], axis=0
    )
    return out.astype(np.float32)


# revision 19
# speedup vs baseline: 1.1774x; 1.0085x over previous
"""Trainium2 Bass kernel: per-pixel channel shuffle + 3x3 conv (stride 1, pad 1).

Problem: x [32,256,56,56] f32, w [256,256,3,3] f32 (OIHW), perm [3136,256] i32;
out[b,:,h,w] = conv3x3(xs)[b,:,h,w] where xs[b,:,l] = x[b, perm[l,:], l].

Strategy (8 NeuronCores, data-parallel over batch, 4 batches/core), using
width-only F(2,3) Winograd (2 output cols from 4 input cols, 4 muls vs 6):
  host: pre-transpose x to [B, HW, C] bf16; build inverse-perm int16 tables
        batched 4 pixel-tiles per GPSIMD local_scatter call; transform w into
        48 Winograd lhsT tiles U[i,kh]; build the 112x224 B-transform matrix
        T that maps a 112-pixel tile (2 image rows) to its 4 Winograd input
        planes x 2 rows x 28 tile-cols (boundary columns dropped = padding).
  device, per batch (pipelined across engines):
    step s (7 per batch): GPSIMD local_scatter shuffles 448 pixels x 256 ch;
    PE multiplies each [112,128] block by T — transposing back to channel-
    major AND computing the Winograd input transform in one matmul -> PSUM;
    Scalar copies the V planes to SBUF. Conv for row-group g=s-2: 24 matmuls
    (4 wino-i x 3 kh x 2 ic-tiles) of N=224 accumulate into 4 PSUM slices
    M0..M3; Scalar stages M1; DVE computes Y_even = (M0+M1)+M2 and
    Y_odd = (M1-M2)-M3 into the w-interleaved staging tile, DMA'd out.
"""

import os
import sys
import types
import numpy as np

_STATE = {}
LAST_RESULT = None

B, C, H, W = 32, 256, 56, 56
HW = H * W
TL = 112
NT = 28
NSC = 7          # scatter steps per batch (4 pixel-tiles each)
K4 = 4           # pixel-tiles per local_scatter call
VPL = 58 * 28    # one V plane: 58 padded rows x 28 tile-cols
N_CORES = 8
B_LOC = B // N_CORES
# Winograd i -> V plane slot (memory order i0,i3,i1,i2 — legacy of the DVE
# variant; any consistent order works since T and the conv agree on it).
PLANE = {0: 0, 3: 1, 1: 2, 2: 3}


def _install_ntff_shim():
    # antenv.axon_hooks is absent in some images; provide it so trace=True
    # (BASS_TRACE=1) can capture NTFF profiles instead of crashing.
    name = "antenv.axon_hooks"
    if name in sys.modules:
        return
    try:
        import antenv  # noqa: F401

        m = types.ModuleType(name)
        m._hook = None
        m.set_axon_ntff_profile_hook = lambda h: setattr(m, "_hook", h)
        m.get_axon_ntff_profile_hook = lambda: m._hook
        sys.modules[name] = m
        setattr(sys.modules["antenv"], "axon_hooks", m)
        from trn_agent_boot.trn_boot import _ntff_profile_via_ctypes

        hook = _ntff_profile_via_ctypes("/opt/axon/libaxon_pjrt.so")
        if hook is not None:
            m.set_axon_ntff_profile_hook(hook)
    except Exception:
        pass


def _build_kernel():
    import concourse.bass as bass
    import concourse.mybir as mybir
    from concourse import bacc, tile
    from contextlib import ExitStack

    F32 = mybir.dt.float32
    BF16 = mybir.dt.bfloat16
    I16 = mybir.dt.int16

    nc = bacc.Bacc("TRN2", target_bir_lowering=False, debug=False, num_devices=N_CORES)

    xb = nc.dram_tensor("xb", [B_LOC, HW, C], BF16, kind="ExternalInput")
    wt = nc.dram_tensor("wt", [48, 128, 128], BF16, kind="ExternalInput")
    tmat = nc.dram_tensor("tmat", [TL, 224], BF16, kind="ExternalInput")
    warm = nc.dram_tensor("warm", [16, 2], I16, kind="ExternalInput")
    idxt = nc.dram_tensor("idxt", [128, NSC * K4 * 256], I16, kind="ExternalInput")
    out = nc.dram_tensor("out", [B_LOC, C, HW], F32, kind="ExternalOutput")

    def sub_ap(ap, off, dims):
        # hand-built sub-AP of an SBUF/PSUM tile slice: keep the partition
        # dim, replace the free dims.
        return bass.AP(ap.tensor, ap.offset + off, [ap.ap[0]] + dims)

    with tile.TileContext(nc) as tc, ExitStack() as ctx:
        const = ctx.enter_context(tc.tile_pool(name="const", bufs=1))
        wsb = const.tile([128, 48 * 128], BF16)
        idxsb = const.tile([128, NSC * 1024], I16)
        tsb = const.tile([128, 224], BF16)
        warmsb = const.tile([16, 4], I16)
        warmout = const.tile([16, 4], BF16)
        nc.sync.dma_start(out=warmsb[0:16, 0:2], in_=warm[:, :])
        nc.sync.dma_start(out=idxsb[:, 0:1024], in_=idxt[:, 0:1024])
        nc.sync.dma_start(out=tsb[0:TL, :], in_=tmat[:, :])
        nc.scalar.dma_start(
            out=wsb[:, :],
            in_=bass.AP(wt, 0, [[128, 128], [128 * 128, 48], [1, 128]]),
        )
        # tiny warm-up scatter: triggers the GPSIMD ext-isa IRAM library
        # load (~6us) while the input DMAs stream in
        nc.gpsimd.local_scatter(
            out_ap=warmout[0:16, 0:2],
            data_ap=warmsb.bitcast(BF16)[0:16, 0:2],
            idxs_ap=warmsb[0:16, 0:2],
            channels=16,
            num_elems=2,
            num_idxs=2,
        )

        xin_pool = ctx.enter_context(tc.tile_pool(name="xin", bufs=2))
        v_pool = ctx.enter_context(tc.tile_pool(name="vv", bufs=2))
        sout_pool = ctx.enter_context(tc.tile_pool(name="sout", bufs=6))
        outst_pool = ctx.enter_context(tc.tile_pool(name="outst", bufs=4))
        mst_pool = ctx.enter_context(tc.tile_pool(name="mst", bufs=4))
        tsc_pool = ctx.enter_context(tc.tile_pool(name="tsc", bufs=4))
        tps_pool = ctx.enter_context(tc.tile_pool(name="tps", bufs=1, space="PSUM"))
        mpsum_pool = ctx.enter_context(tc.tile_pool(name="mpsum", bufs=2, space="PSUM"))

        xin_t = [None] * B_LOC

        def issue_xin_dma(b, chunks=1):
            xin_t[b] = xin_pool.tile([128, NT * 256], BF16, name="xin", tag="xin")
            step = NSC // chunks
            for s in range(0, NSC, step):
                nc.sync.dma_start(
                    out=xin_t[b][0:TL, s * 1024 : (s + step) * 1024],
                    in_=bass.AP(
                        xb,
                        b * HW * C + s * K4 * TL * 256,
                        [[256, TL], [TL * 256, step * K4], [1, 256]],
                    ),
                )

        def wconv_group(b, vt, oct, g):
            mp = mpsum_pool.tile([128, 1024], F32)
            for i in range(4):
                for kh in range(3):
                    for ct in range(2):
                        widx = ((i * 3 + kh) * 2 + ct) * 2 + oct
                        voff = (ct * 4 + PLANE[i]) * VPL + (8 * g + kh) * 28
                        nc.tensor.matmul(
                            mp[:, i * 256 : i * 256 + 224],
                            lhsT=wsb[:, widx * 128 : (widx + 1) * 128],
                            rhs=vt[:, voff : voff + 224],
                            start=(kh == 0 and ct == 0),
                            stop=(kh == 2 and ct == 1),
                        )
            # stage M1 to SBUF (bf16) so every DVE inverse op touches at
            # most one PSUM operand (ISA limit).
            ms = mst_pool.tile([128, 224], BF16)
            nc.scalar.copy(ms[:, :], mp[:, 256:480])
            t01 = tsc_pool.tile([128, 448], BF16)
            nc.vector.tensor_add(t01[:, 0:224], mp[:, 0:224], ms[:, :])
            nc.vector.tensor_sub(t01[:, 224:448], ms[:, :], mp[:, 512:736])
            ost = outst_pool.tile([128, 448], F32)
            oap = ost[:, :]
            even = sub_ap(oap, 0, [[56, 8], [2, 28]])
            odd = sub_ap(oap, 1, [[56, 8], [2, 28]])
            r28 = lambda ap: ap.rearrange("p (r c) -> p r c", r=8)
            nc.vector.tensor_add(even, r28(t01[:, 0:224]), r28(mp[:, 512:736]))
            nc.vector.tensor_sub(odd, r28(t01[:, 224:448]), r28(mp[:, 768:992]))
            nc.sync.dma_start(
                out=out[b, oct * 128 : (oct + 1) * 128, g * 448 : (g + 1) * 448],
                in_=ost[:, :],
            )

        issue_xin_dma(0, chunks=7)
        for s in range(1, NSC):
            nc.sync.dma_start(
                out=idxsb[:, s * 1024 : (s + 1) * 1024],
                in_=idxt[:, s * 1024 : (s + 1) * 1024],
            )
        for b in range(B_LOC):
            if b + 1 < B_LOC:
                issue_xin_dma(b + 1)
            xin = xin_t[b]

            vt = v_pool.tile([128, 8 * VPL], BF16, name="vt", tag="vt")
            va = vt[:, :]
            # top/bottom padding rows of every V plane are zero
            for ct in range(2):
                for row in (0, 57):
                    nc.vector.memset(
                        sub_ap(va, ct * 4 * VPL + row * 28, [[VPL, 4], [1, 28]]),
                        0.0,
                    )

            for s in range(NSC):
                if s >= 2:
                    wconv_group(b, vt, 0, s - 2)
                    wconv_group(b, vt, 1, s - 2)
                sout = sout_pool.tile([128, K4 * 256], BF16, name="sout", tag="sout")
                nc.gpsimd.local_scatter(
                    out_ap=sout[0:TL, :],
                    data_ap=xin[0:TL, s * 1024 : (s + 1) * 1024],
                    idxs_ap=idxsb[0:TL, s * 1024 : (s + 1) * 1024],
                    channels=TL,
                    num_elems=K4 * 256,
                    num_idxs=K4 * 256,
                )
                # per [112,128] block: transpose-back x Winograd-B in one
                # matmul into f32 PSUM; block j = ct*4+k at a bank-safe
                # offset (two 224-col blocks per 512-f32 bank)
                ps2 = tps_pool.tile([128, 2048], F32, name="ps2", tag="ps2")
                for ct in range(2):
                    for k in range(K4):
                        j = ct * K4 + k
                        off = (j // 2) * 512 + (j % 2) * 224
                        nc.tensor.matmul(
                            ps2[:, off : off + 224],
                            lhsT=sout[0:TL, k * 256 + ct * 128 : k * 256 + ct * 128 + 128],
                            rhs=tsb[0:TL, :],
                            start=True,
                            stop=True,
                        )
                # copy the step's V rows (8s+1 .. 8s+8) into the V planes,
                # one copy per (ct, pair-of-tiles)
                pa = ps2[:, :]
                for ct in range(2):
                    for k2 in range(2):
                        nc.scalar.copy(
                            sub_ap(
                                va,
                                ct * 4 * VPL + (8 * s + 1 + 4 * k2) * 28,
                                [[VPL, 4], [56, 2], [28, 2], [1, 28]],
                            ),
                            sub_ap(
                                pa,
                                ct * 1024 + k2 * 512,
                                [[56, 4], [224, 2], [28, 2], [1, 28]],
                            ),
                        )

            for g in (5, 6):
                wconv_group(b, vt, 0, g)
                wconv_group(b, vt, 1, g)

    nc.compile()
    return nc


def _host_prep(x, w, perm):
    import ml_dtypes

    # [B, C, H, W] -> [B, HW, C] bf16 (pixel-major so scatter tiles DMA
    # straight into [pixel, channel] layout)
    xf = np.ascontiguousarray(
        x.reshape(B, C, HW).transpose(0, 2, 1)
    ).astype(ml_dtypes.bfloat16)

    # F(2,3) width-Winograd weight transform: U0 = g0, U1 = (g0+g1+g2)/2,
    # U2 = (g0-g1+g2)/2, U3 = g2 per (kh, ic-tile, oc-tile), stored as lhsT.
    wf = np.asarray(w, dtype=np.float32)
    wtl = np.empty((48, 128, 128), dtype=ml_dtypes.bfloat16)
    for ct in range(2):
        for kh in range(3):
            for oct in range(2):
                blk = wf[
                    oct * 128 : (oct + 1) * 128, ct * 128 : (ct + 1) * 128, kh, :
                ]  # [oc, ic, 3]
                g0, g1, g2 = blk[:, :, 0], blk[:, :, 1], blk[:, :, 2]
                U = [g0, 0.5 * (g0 + g1 + g2), 0.5 * (g0 - g1 + g2), g2]
                for i in range(4):
                    widx = ((i * 3 + kh) * 2 + ct) * 2 + oct
                    wtl[widx] = U[i].T.astype(ml_dtypes.bfloat16)

    # B-transform matrix: block pixel p = (r_loc, w); output col =
    # plane*56 + r_loc*28 + tc. V[i][tc] = sum_a Bcoef[i][a] * d(2tc+a-1)
    # where d is the image column (pad columns outside [0,56) drop out).
    BCOEF = {0: {0: 1.0, 2: -1.0}, 1: {1: 1.0, 2: 1.0},
             2: {2: 1.0, 1: -1.0}, 3: {1: 1.0, 3: -1.0}}
    tmat = np.zeros((TL, 224), dtype=np.float32)
    for i in range(4):
        pl = PLANE[i]
        for tc in range(28):
            for a, cf in BCOEF[i].items():
                wcol = 2 * tc + a - 1
                if 0 <= wcol < 56:
                    for r_loc in range(2):
                        tmat[r_loc * 56 + wcol, pl * 56 + r_loc * 28 + tc] = cf
    tmat = tmat.astype(ml_dtypes.bfloat16)

    iperm = np.empty((HW, C), dtype=np.int16)
    np.put_along_axis(
        iperm, perm.astype(np.int64), np.arange(C, dtype=np.int16)[None, :], axis=1
    )
    idxt = np.zeros((128, NSC * 1024), dtype=np.int16)
    for s in range(NSC):
        for k in range(K4):
            t = K4 * s + k
            idxt[0:TL, s * 1024 + k * 256 : s * 1024 + (k + 1) * 256] = (
                iperm[t * TL : (t + 1) * TL, :] + k * 256
            )

    in_maps = []
    for cidx in range(N_CORES):
        in_maps.append(
            {
                "xb": np.ascontiguousarray(xf[cidx * B_LOC : (cidx + 1) * B_LOC]),
                "wt": wtl,
                "tmat": tmat,
                "warm": np.tile(np.array([0, 1], dtype=np.int16), (16, 1)),
                "idxt": idxt,
            }
        )
    return in_maps


def kernel(x, w, perm):
    global LAST_RESULT
    _install_ntff_shim()
    from concourse.bass_utils import run_bass_kernel_spmd

    x = np.asarray(x, dtype=np.float32)
    w = np.asarray(w, dtype=np.float32)
    perm = np.asarray(perm)

    if "nc" not in _STATE:
        _STATE["nc"] = _build_kernel()
    nc = _STATE["nc"]

    in_maps = _host_prep(x, w, perm)
    res = run_bass_kernel_spmd(nc, in_maps, core_ids=list(range(N_CORES)))
    LAST_RESULT = res
    out = np.concatenate(
        [r["out"].reshape(B_LOC, C, H, W) for r in res.results], axis=0
    )
    return out.astype(np.float32)


# revision 21
# speedup vs baseline: 1.3335x; 1.1326x over previous
"""Trainium2 Bass kernel: per-pixel channel shuffle + 3x3 conv (stride 1, pad 1).

Problem: x [32,256,56,56] f32, w [256,256,3,3] f32 (OIHW), perm [3136,256] i32;
out[b,:,h,w] = conv3x3(xs)[b,:,h,w] where xs[b,:,l] = x[b, perm[l,:], l].

Strategy (8 NeuronCores, data-parallel over batch, 4 batches/core), using
width-only F(2,3) Winograd (2 output cols from 4 input cols, 4 muls vs 6):
  host: pre-transpose x to [B, HW, C] bf16; build inverse-perm int16 tables
        batched 4 pixel-tiles per GPSIMD local_scatter call; transform w into
        48 Winograd lhsT tiles U[i,kh]; build the 112x224 B-transform matrix
        T that maps a 112-pixel tile (2 image rows) to its 4 Winograd input
        planes x 2 rows x 28 tile-cols (boundary columns dropped = padding).
  device, per batch (pipelined across engines):
    step s (7 per batch): GPSIMD local_scatter shuffles 448 pixels x 256 ch;
    PE multiplies each [112,128] block by T — transposing back to channel-
    major AND computing the Winograd input transform in one matmul -> PSUM;
    Scalar copies the V planes to SBUF. Conv for row-group g=s-2: 24 matmuls
    (4 wino-i x 3 kh x 2 ic-tiles) of N=224 accumulate into 4 PSUM slices
    M0..M3; Scalar stages M1; DVE computes Y_even = (M0+M1)+M2 and
    Y_odd = (M1-M2)-M3 into the w-interleaved staging tile, DMA'd out.
"""

import os
import sys
import types
import numpy as np

_STATE = {}
LAST_RESULT = None

B, C, H, W = 32, 256, 56, 56
HW = H * W
TL = 112
NT = 28
NSC = 7          # scatter steps per batch (4 pixel-tiles each)
K4 = 4           # pixel-tiles per local_scatter call
VPL = 58 * 28    # one V plane: 58 padded rows x 28 tile-cols
N_CORES = 8
B_LOC = B // N_CORES
# Winograd i -> V plane slot (memory order i0,i3,i1,i2 — legacy of the DVE
# variant; any consistent order works since T and the conv agree on it).
PLANE = {0: 0, 3: 1, 1: 2, 2: 3}


def _install_ntff_shim():
    # antenv.axon_hooks is absent in some images; provide it so trace=True
    # (BASS_TRACE=1) can capture NTFF profiles instead of crashing.
    name = "antenv.axon_hooks"
    if name in sys.modules:
        return
    try:
        import antenv  # noqa: F401

        m = types.ModuleType(name)
        m._hook = None
        m.set_axon_ntff_profile_hook = lambda h: setattr(m, "_hook", h)
        m.get_axon_ntff_profile_hook = lambda: m._hook
        sys.modules[name] = m
        setattr(sys.modules["antenv"], "axon_hooks", m)
        from trn_agent_boot.trn_boot import _ntff_profile_via_ctypes

        hook = _ntff_profile_via_ctypes("/opt/axon/libaxon_pjrt.so")
        if hook is not None:
            m.set_axon_ntff_profile_hook(hook)
    except Exception:
        pass


def _build_kernel():
    import concourse.bass as bass
    import concourse.mybir as mybir
    from concourse import bacc, tile
    from contextlib import ExitStack

    F32 = mybir.dt.float32
    BF16 = mybir.dt.bfloat16
    I16 = mybir.dt.int16

    nc = bacc.Bacc("TRN2", target_bir_lowering=False, debug=False, num_devices=N_CORES)

    xb = nc.dram_tensor("xb", [B_LOC, HW, C], BF16, kind="ExternalInput")
    wt = nc.dram_tensor("wt", [48, 128, 128], BF16, kind="ExternalInput")
    tmat = nc.dram_tensor("tmat", [TL, 224], BF16, kind="ExternalInput")
    warm = nc.dram_tensor("warm", [16, 2], I16, kind="ExternalInput")
    idxt = nc.dram_tensor("idxt", [128, NSC * K4 * 256], I16, kind="ExternalInput")
    out = nc.dram_tensor("out", [B_LOC, C, HW], F32, kind="ExternalOutput")

    def sub_ap(ap, off, dims):
        # hand-built sub-AP of an SBUF/PSUM tile slice: keep the partition
        # dim, replace the free dims.
        return bass.AP(ap.tensor, ap.offset + off, [ap.ap[0]] + dims)

    with tile.TileContext(nc) as tc, ExitStack() as ctx:
        const = ctx.enter_context(tc.tile_pool(name="const", bufs=1))
        wsb = const.tile([128, 48 * 128], BF16)
        idxsb = const.tile([128, NSC * 1024], I16)
        tsb = const.tile([128, 224], BF16)
        warmsb = const.tile([16, 4], I16)
        warmout = const.tile([16, 4], BF16)
        nc.sync.dma_start(out=warmsb[0:16, 0:2], in_=warm[:, :])
        nc.sync.dma_start(out=idxsb[:, 0:1024], in_=idxt[:, 0:1024])
        nc.sync.dma_start(out=tsb[0:TL, :], in_=tmat[:, :])
        nc.scalar.dma_start(
            out=wsb[:, :],
            in_=bass.AP(wt, 0, [[128, 128], [128 * 128, 48], [1, 128]]),
        )
        # tiny warm-up scatter: triggers the GPSIMD ext-isa IRAM library
        # load (~6us) while the input DMAs stream in
        nc.gpsimd.local_scatter(
            out_ap=warmout[0:16, 0:2],
            data_ap=warmsb.bitcast(BF16)[0:16, 0:2],
            idxs_ap=warmsb[0:16, 0:2],
            channels=16,
            num_elems=2,
            num_idxs=2,
        )

        xin_pool = ctx.enter_context(tc.tile_pool(name="xin", bufs=2))
        v_pool = ctx.enter_context(tc.tile_pool(name="vv", bufs=2))
        sout_pool = ctx.enter_context(tc.tile_pool(name="sout", bufs=6))
        outst_pool = ctx.enter_context(tc.tile_pool(name="outst", bufs=4))
        mst_pool = ctx.enter_context(tc.tile_pool(name="mst", bufs=4))
        tsc_pool = ctx.enter_context(tc.tile_pool(name="tsc", bufs=4))
        tps_pool = ctx.enter_context(tc.tile_pool(name="tps", bufs=1, space="PSUM"))
        mpsum_pool = ctx.enter_context(tc.tile_pool(name="mpsum", bufs=2, space="PSUM"))

        xin_t = [None] * B_LOC

        def issue_xin_dma(b, chunks=1):
            xin_t[b] = xin_pool.tile([128, NT * 256], BF16, name="xin", tag="xin")
            step = NSC // chunks
            for s in range(0, NSC, step):
                nc.sync.dma_start(
                    out=xin_t[b][0:TL, s * 1024 : (s + step) * 1024],
                    in_=bass.AP(
                        xb,
                        b * HW * C + s * K4 * TL * 256,
                        [[256, TL], [TL * 256, step * K4], [1, 256]],
                    ),
                )

        def wconv_group(b, vt, oct, g):
            mp = mpsum_pool.tile([128, 1024], F32)
            for i in range(4):
                for kh in range(3):
                    for ct in range(2):
                        widx = ((i * 3 + kh) * 2 + ct) * 2 + oct
                        voff = (ct * 4 + PLANE[i]) * VPL + (8 * g + kh) * 28
                        nc.tensor.matmul(
                            mp[:, i * 256 : i * 256 + 224],
                            lhsT=wsb[:, widx * 128 : (widx + 1) * 128],
                            rhs=vt[:, voff : voff + 224],
                            start=(kh == 0 and ct == 0),
                            stop=(kh == 2 and ct == 1),
                        )
            # stage M1 to SBUF (bf16) so every DVE inverse op touches at
            # most one PSUM operand (ISA limit).
            ms = mst_pool.tile([128, 224], BF16)
            nc.scalar.copy(ms[:, :], mp[:, 256:480])
            t01 = tsc_pool.tile([128, 448], BF16)
            nc.vector.tensor_add(t01[:, 0:224], mp[:, 0:224], ms[:, :])
            nc.vector.tensor_sub(t01[:, 224:448], ms[:, :], mp[:, 512:736])
            ost = outst_pool.tile([128, 448], F32)
            oap = ost[:, :]
            even = sub_ap(oap, 0, [[56, 8], [2, 28]])
            odd = sub_ap(oap, 1, [[56, 8], [2, 28]])
            r28 = lambda ap: ap.rearrange("p (r c) -> p r c", r=8)
            nc.vector.tensor_add(even, r28(t01[:, 0:224]), r28(mp[:, 512:736]))
            nc.vector.tensor_sub(odd, r28(t01[:, 224:448]), r28(mp[:, 768:992]))
            nc.sync.dma_start(
                out=out[b, oct * 128 : (oct + 1) * 128, g * 448 : (g + 1) * 448],
                in_=ost[:, :],
            )

        issue_xin_dma(0, chunks=7)
        for s in range(1, NSC):
            nc.sync.dma_start(
                out=idxsb[:, s * 1024 : (s + 1) * 1024],
                in_=idxt[:, s * 1024 : (s + 1) * 1024],
            )
        for b in range(B_LOC):
            if b + 1 < B_LOC:
                issue_xin_dma(b + 1)
            xin = xin_t[b]

            vt = v_pool.tile([128, 8 * VPL], BF16, name="vt", tag="vt")
            va = vt[:, :]
            # top/bottom padding rows of every V plane are zero
            for ct in range(2):
                for row in (0, 57):
                    nc.vector.memset(
                        sub_ap(va, ct * 4 * VPL + row * 28, [[VPL, 4], [1, 28]]),
                        0.0,
                    )

            for s in range(NSC):
                sout = sout_pool.tile([128, K4 * 256], BF16, name="sout", tag="sout")
                nc.gpsimd.local_scatter(
                    out_ap=sout[0:TL, :],
                    data_ap=xin[0:TL, s * 1024 : (s + 1) * 1024],
                    idxs_ap=idxsb[0:TL, s * 1024 : (s + 1) * 1024],
                    channels=TL,
                    num_elems=K4 * 256,
                    num_idxs=K4 * 256,
                )
                # per [112,128] block: transpose-back x Winograd-B in one
                # matmul into f32 PSUM; block j = ct*4+k at a bank-safe
                # offset (two 224-col blocks per 512-f32 bank)
                ps2 = tps_pool.tile([128, 2048], F32, name="ps2", tag="ps2")
                for ct in range(2):
                    for k in range(K4):
                        j = ct * K4 + k
                        off = (j // 2) * 512 + (j % 2) * 224
                        nc.tensor.matmul(
                            ps2[:, off : off + 224],
                            lhsT=sout[0:TL, k * 256 + ct * 128 : k * 256 + ct * 128 + 128],
                            rhs=tsb[0:TL, :],
                            start=True,
                            stop=True,
                        )
                # copy the step's V rows (8s+1 .. 8s+8) into the V planes,
                # one copy per (ct, pair-of-tiles)
                pa = ps2[:, :]
                for ct in range(2):
                    for k2 in range(2):
                        nc.scalar.copy(
                            sub_ap(
                                va,
                                ct * 4 * VPL + (8 * s + 1 + 4 * k2) * 28,
                                [[VPL, 4], [56, 2], [28, 2], [1, 28]],
                            ),
                            sub_ap(
                                pa,
                                ct * 1024 + k2 * 512,
                                [[56, 4], [224, 2], [28, 2], [1, 28]],
                            ),
                        )
                # conv LAST in the step: its V rows were copied during the
                # previous step's conv, so the PE never waits on Scalar
                if s >= 2:
                    wconv_group(b, vt, 0, s - 2)
                    wconv_group(b, vt, 1, s - 2)

            for g in (5, 6):
                wconv_group(b, vt, 0, g)
                wconv_group(b, vt, 1, g)

    nc.compile()
    return nc


def _host_prep(x, w, perm):
    import ml_dtypes

    # [B, C, H, W] -> [B, HW, C] bf16 (pixel-major so scatter tiles DMA
    # straight into [pixel, channel] layout)
    xf = np.ascontiguousarray(
        x.reshape(B, C, HW).transpose(0, 2, 1)
    ).astype(ml_dtypes.bfloat16)

    # F(2,3) width-Winograd weight transform: U0 = g0, U1 = (g0+g1+g2)/2,
    # U2 = (g0-g1+g2)/2, U3 = g2 per (kh, ic-tile, oc-tile), stored as lhsT.
    wf = np.asarray(w, dtype=np.float32)
    wtl = np.empty((48, 128, 128), dtype=ml_dtypes.bfloat16)
    for ct in range(2):
        for kh in range(3):
            for oct in range(2):
                blk = wf[
                    oct * 128 : (oct + 1) * 128, ct * 128 : (ct + 1) * 128, kh, :
                ]  # [oc, ic, 3]
                g0, g1, g2 = blk[:, :, 0], blk[:, :, 1], blk[:, :, 2]
                U = [g0, 0.5 * (g0 + g1 + g2), 0.5 * (g0 - g1 + g2), g2]
                for i in range(4):
                    widx = ((i * 3 + kh) * 2 + ct) * 2 + oct
                    wtl[widx] = U[i].T.astype(ml_dtypes.bfloat16)

    # B-transform matrix: block pixel p = (r_loc, w); output col =
    # plane*56 + r_loc*28 + tc. V[i][tc] = sum_a Bcoef[i][a] * d(2tc+a-1)
    # where d is the image column (pad columns outside [0,56) drop out).
    BCOEF = {0: {0: 1.0, 2: -1.0}, 1: {1: 1.0, 2: 1.0},
             2: {2: 1.0, 1: -1.0}, 3: {1: 1.0, 3: -1.0}}
    tmat = np.zeros((TL, 224), dtype=np.float32)
    for i in range(4):
        pl = PLANE[i]
        for tc in range(28):
            for a, cf in BCOEF[i].items():
                wcol = 2 * tc + a - 1
                if 0 <= wcol < 56:
                    for r_loc in range(2):
                        tmat[r_loc * 56 + wcol, pl * 56 + r_loc * 28 + tc] = cf
    tmat = tmat.astype(ml_dtypes.bfloat16)

    iperm = np.empty((HW, C), dtype=np.int16)
    np.put_along_axis(
        iperm, perm.astype(np.int64), np.arange(C, dtype=np.int16)[None, :], axis=1
    )
    idxt = np.zeros((128, NSC * 1024), dtype=np.int16)
    for s in range(NSC):
        for k in range(K4):
            t = K4 * s + k
            idxt[0:TL, s * 1024 + k * 256 : s * 1024 + (k + 1) * 256] = (
                iperm[t * TL : (t + 1) * TL, :] + k * 256
            )

    in_maps = []
    for cidx in range(N_CORES):
        in_maps.append(
            {
                "xb": np.ascontiguousarray(xf[cidx * B_LOC : (cidx + 1) * B_LOC]),
                "wt": wtl,
                "tmat": tmat,
                "warm": np.tile(np.array([0, 1], dtype=np.int16), (16, 1)),
                "idxt": idxt,
            }
        )
    return in_maps


def kernel(x, w, perm):
    global LAST_RESULT
    _install_ntff_shim()
    from concourse.bass_utils import run_bass_kernel_spmd

    x = np.asarray(x, dtype=np.float32)
    w = np.asarray(w, dtype=np.float32)
    perm = np.asarray(perm)

    if "nc" not in _STATE:
        _STATE["nc"] = _build_kernel()
    nc = _STATE["nc"]

    in_maps = _host_prep(x, w, perm)
    res = run_bass_kernel_spmd(nc, in_maps, core_ids=list(range(N_CORES)))
    LAST_RESULT = res
    out = np.concatenate(
        [r["out"].reshape(B_LOC, C, H, W) for r in res.results], axis=0
    )
    return out.astype(np.float32)
